# revision 17
# baseline (speedup 1.0000x reference)
"""Trainium2 Bass kernel for nn_Damping (tiny 2->16->16->{2,1} tanh MLPs + 2x2 LL^T).

Strategy: pure data parallel over 8 cores (batch split). Inside each core:
  - x is host-packed into a "4-slot" layout: tiles of [8, 512] where rows are
    (slot s, coord c) and each slot covers 512 consecutive samples. This lets
    layers 1/2 run as block-diagonal matmuls producing [128, 512] tiles
    (4 slots x 32 features) that keep the ACT/DVE lanes full.
  - Matmul operands are fp16 (10-bit mantissa, same as TF32): 1 cycle/row on
    the PE (fp32 matmuls lower to 2 half-rate passes = 4x slower) and 2-byte
    fast weight loads. PSUM accumulation stays fp32; measured rel l2 error
    vs the fp32 reference is ~6e-4.
  - tanh+bias is fused on the scalar engine reading PSUM directly, one
    [128, 1024] ACTIVATE per layer per 2-tile chunk (amortizes the ~352-cycle
    per-instruction overhead; PSUM: h1p 2x2 banks, h2p 2, y 2).
  - Layer 3 is "flipped": lhsT = h2 chunk (stationary), rhs = packed w3, so
    the output lands batch-major in PSUM. A rank-1 bias matmul opens each
    4-chunk "quad" accumulation group (start=True clears the bank's
    has_written bits before the 32 layer-3 matmuls accumulate in).
  - The relu/damping/LL^T epilogue runs once per quad (16384 samples) on the
    vector engine with [128, 128] strided views.
Host packs/unpacks with pure numpy (layout marshalling only; all math on HW).
Measured on trn2: ~345 us/core HW exec for 4M samples (8 cores).
"""

import numpy as np
from contextlib import ExitStack

import concourse.bass as bass
import concourse.bacc as bacc
import concourse.tile as tile
import concourse.mybir as mybir
from concourse.bass_utils import run_bass_kernel_spmd

F32 = mybir.dt.float32
F32R = mybir.dt.float32r
F16 = mybir.dt.float16
AF = mybir.ActivationFunctionType
ALU = mybir.AluOpType

NCORES = 8
N = 512            # samples per slot per tile (one PSUM bank)
TS = 4 * N         # samples per tile (4 slots stacked -> 128 partitions)
CHUNK = 2          # tiles per supergroup (PSUM: 2x2+2+2 banks)
DAMP = 1e-3

_programs = {}


def _chunks(G):
    out = []
    g = 0
    while g < G:
        n = min(CHUNK, G - g)
        out.append((g, n))
        g += n
    return out


def build_program(S):
    """One-core program; SPMD across cores via per-core inputs.

    Loop structure: quads of 4 chunks (chunk = 2 tiles of [8,512] -> 4096
    samples). PSUM: h1p [128,1024] bufs=2 (4 banks) + h2p bufs=1 (2 banks) +
    y-quad [128,384] bufs=2 (2 banks). The layer-3 bias matmul opens each
    quad's accumulation group (start=True clears the bank), the 32 flipped
    layer-3 matmuls accumulate into it, and the epilogue+store run once per
    quad on full-width [128,128] views.
    """
    G = S // TS            # [8,512] tiles
    NQ = G // 8            # quads (8 tiles each)
    assert S % TS == 0 and G % 8 == 0
    nc = bacc.Bacc(None, target_bir_lowering=False)
    xp_d = nc.declare_dram_parameter("xp", [8, G * N], F16, isOutput=False)
    xe_d = nc.declare_dram_parameter("xe", [128, G, 4, 4, 2], F32, isOutput=False)
    w1_d = nc.declare_dram_parameter("w1", [8, 128], F16, isOutput=False)
    w2_d = nc.declare_dram_parameter("w2", [128, 128], F16, isOutput=False)
    w3_d = nc.declare_dram_parameter("w3", [128, 12], F16, isOutput=False)
    b1_d = nc.declare_dram_parameter("b1", [128, 1], F32, isOutput=False)
    b2_d = nc.declare_dram_parameter("b2", [128, 1], F32, isOutput=False)
    b3_d = nc.declare_dram_parameter("b3", [1, 384], F16, isOutput=False)
    out_d = nc.declare_dram_parameter("out", [128, G, 4, 4, 2], F32, isOutput=True)

    with tile.TileContext(nc) as tc:
        with ExitStack() as ctx:
            consts = ctx.enter_context(tc.tile_pool(name="consts", bufs=1))
            w1 = consts.tile([8, 128], F16)
            w2 = consts.tile([128, 128], F16)
            w3 = consts.tile([128, 12], F16)
            b1 = consts.tile([128, 1], F32)
            b2 = consts.tile([128, 1], F32)
            b3 = consts.tile([1, 384], F16)
            ones = consts.tile([1, 128], F16)
            nc.sync.dma_start(w1[:], w1_d[:])
            nc.sync.dma_start(w2[:], w2_d[:])
            nc.sync.dma_start(w3[:], w3_d[:])
            nc.sync.dma_start(b1[:], b1_d[:])
            nc.sync.dma_start(b2[:], b2_d[:])
            nc.sync.dma_start(b3[:], b3_d[:])
            nc.vector.memset(ones[:], 1.0)

            xp_pool = ctx.enter_context(tc.tile_pool(name="xp", bufs=3))
            xe_pool = ctx.enter_context(tc.tile_pool(name="xe", bufs=3))
            h_pool = ctx.enter_context(tc.tile_pool(name="h", bufs=4))
            o_pool = ctx.enter_context(tc.tile_pool(name="o", bufs=3))
            t_pool = ctx.enter_context(tc.tile_pool(name="t", bufs=3))
            ps1 = ctx.enter_context(
                tc.tile_pool(name="ps1", bufs=2, space=bass.MemorySpace.PSUM))
            ps2 = ctx.enter_context(
                tc.tile_pool(name="ps2", bufs=1, space=bass.MemorySpace.PSUM))
            psy = ctx.enter_context(
                tc.tile_pool(name="psy", bufs=2, space=bass.MemorySpace.PSUM))

            NC = 4 * NQ                       # chunks of 2 tiles
            quad_res = {}
            for c in range(NC + 1):
                if c < NC:
                    q, qc = divmod(c, 4)
                    if qc == 0:
                        g0 = q * 8
                        xp_t = xp_pool.tile([8, 8 * N], F16, tag="xp")
                        nc.sync.dma_start(xp_t[:], xp_d[:, g0 * N:(g0 + 8) * N])
                        xe_t = xe_pool.tile([128, 8, 4, 4, 2], F32, tag="xe")
                        nc.sync.dma_start(xe_t[:], xe_d[:, g0:g0 + 8])
                        y = psy.tile([128, 8, 4, 4, 3], F32, tag="y")
                        # opens the quad's accumulation group: clears the
                        # bank's has_written bits, deposits layer-3 biases
                        nc.tensor.matmul(y[:], ones[:], b3[:], start=True,
                                         stop=False, skip_group_check=True)
                        quad_res[q] = (xp_t, xe_t, y)
                    xp_t, _, _ = quad_res[q]
                    c0 = 2 * qc
                    h1p = ps1.tile([128, 2 * N], F32, tag="h1p")
                    for t in range(2):
                        nc.tensor.matmul(h1p[:, t * N:(t + 1) * N], w1[:],
                                         xp_t[:, (c0 + t) * N:(c0 + t + 1) * N],
                                         start=True, stop=True)
                    pending = (c, h1p)
                if c >= 1:
                    cp, h1p_p = prev_pending
                    qp, qcp = divmod(cp, 4)
                    _, xe_p, y_p = quad_res[qp]
                    c0 = 2 * qcp
                    h1 = h_pool.tile([128, 2 * N], F16, tag="h1")
                    nc.scalar.activation(h1[:], h1p_p[:], AF.Tanh, bias=b1[:])
                    h2p = ps2.tile([128, 2 * N], F32, tag="h2p")
                    for t in range(2):
                        nc.tensor.matmul(h2p[:, t * N:(t + 1) * N], w2[:],
                                         h1[:, t * N:(t + 1) * N],
                                         start=True, stop=True)
                    h2 = h_pool.tile([128, 2 * N], F16, tag="h2")
                    nc.scalar.activation(h2[:], h2p[:], AF.Tanh, bias=b2[:])
                    for ck in range(8):       # flipped layer 3
                        t, k = divmod(ck, 4)
                        nc.tensor.matmul(y_p[:, c0 + t, k],
                                         h2[:, ck * 128:(ck + 1) * 128],
                                         w3[:], start=False,
                                         stop=(qcp == 3 and ck == 7),
                                         skip_group_check=True)
                    if qcp == 3:              # quad complete: epilogue + store
                        A = y_p[:, :, :, :, 0]
                        Bv = y_p[:, :, :, :, 1]
                        C = y_p[:, :, :, :, 2]
                        X0 = xe_p[:, :, :, :, 0]
                        X1 = xe_p[:, :, :, :, 1]
                        out_t = o_pool.tile([128, 8, 4, 4, 2], F32, tag="out")
                        D0 = out_t[:, :, :, :, 0]
                        D1 = out_t[:, :, :, :, 1]

                        def tmp(tag):
                            return t_pool.tile([128, 8, 4, 4], F32, tag=tag,
                                               name=tag)

                        ra = tmp("ra")
                        nc.vector.tensor_scalar(ra, A, DAMP, DAMP, ALU.add, ALU.max)
                        rb = tmp("rb")
                        nc.vector.tensor_scalar(rb, Bv, DAMP, DAMP, ALU.add, ALU.max)
                        av = tmp("av")
                        nc.vector.tensor_tensor(av, ra, X0, ALU.mult)
                        bv = tmp("bv")
                        nc.vector.tensor_tensor(bv, rb, X1, ALU.mult)
                        u = tmp("u")
                        nc.vector.tensor_tensor(u, av, X0, ALU.mult)
                        v = tmp("v")
                        nc.vector.tensor_tensor(v, C, X1, ALU.mult)
                        w = tmp("w")
                        nc.vector.tensor_tensor(w, u, v, ALU.add)
                        nc.vector.tensor_tensor(D0, av, w, ALU.mult)
                        q_ = tmp("q_")
                        nc.vector.tensor_tensor(q_, bv, bv, ALU.mult)
                        r = tmp("r")
                        nc.vector.tensor_tensor(r, q_, X1, ALU.mult)
                        p1 = tmp("p1")
                        nc.vector.tensor_tensor(p1, C, w, ALU.mult)
                        nc.vector.tensor_tensor(D1, p1, r, ALU.add)
                        nc.sync.dma_start(out_d[:, qp * 8:qp * 8 + 8], out_t[:])
                        del quad_res[qp]
                prev_pending = pending if c < NC else None

    nc.compile()
    return nc


def _get_program(S):
    if S not in _programs:
        _programs[S] = build_program(S)
    return _programs[S]


def _pack_weights(w_d1, w_d2, w_d3, w_o1, w_o2, w_o3,
                  b_d1, b_d2, b_d3, b_o1, b_o2, b_o3):
    W1 = np.concatenate([w_d1, w_o1], axis=1)          # [2, 32]
    W1cat = np.zeros((8, 128), np.float32)
    W2sub = np.zeros((32, 32), np.float32)
    W2sub[:16, :16] = w_d2
    W2sub[16:, 16:] = w_o2
    W2cat = np.zeros((128, 128), np.float32)
    W3sub = np.zeros((32, 3), np.float32)
    W3sub[:16, :2] = w_d3
    W3sub[16:, 2] = w_o3[:, 0]
    W3cat = np.zeros((128, 12), np.float32)
    for s in range(4):
        W1cat[2 * s:2 * s + 2, 32 * s:32 * s + 32] = W1
        W2cat[32 * s:32 * s + 32, 32 * s:32 * s + 32] = W2sub
        W3cat[32 * s:32 * s + 32, 3 * s:3 * s + 3] = W3sub
    bias1 = np.tile(np.concatenate([b_d1, b_o1]), 4).astype(np.float32)[:, None]
    bias2 = np.tile(np.concatenate([b_d2, b_o2]), 4).astype(np.float32)[:, None]
    b3 = np.array([b_d3[0], b_d3[1], b_o3[0]], np.float32)
    b3rep = np.tile(b3, 128)[None, :].astype(np.float16)
    return (W1cat.astype(np.float16), W2cat.astype(np.float16),
            W3cat.astype(np.float16), bias1, bias2, b3rep)


def _pack_x(xc):
    """xc: [S, 2] -> (xp [8, G*N], xe [128, G, 4, 4, 2])."""
    S = xc.shape[0]
    G = S // TS
    xp = np.ascontiguousarray(
        xc.reshape(G, 4, N, 2).transpose(1, 3, 0, 2).reshape(8, G * N)
        .astype(np.float16))
    xe = np.ascontiguousarray(
        xc.reshape(G, 4, 4, 128, 2).transpose(3, 0, 2, 1, 4))
    return xp, xe


def _unpack_out(o):
    """o: [128, G, 4(k), 4(s), 2] -> [S, 2]."""
    G = o.shape[1]
    return np.ascontiguousarray(
        o.transpose(1, 3, 2, 0, 4).reshape(G * TS, 2))


def run(inputs, trace=False, n_cores=NCORES):
    x = np.ascontiguousarray(np.asarray(inputs["x"], np.float32))
    B = x.shape[0]
    S = B // n_cores
    nc = _get_program(S)
    wargs = _pack_weights(
        *(np.asarray(inputs[k], np.float32) for k in
          ("w_d1", "w_d2", "w_d3", "w_o1", "w_o2", "w_o3",
           "b_d1", "b_d2", "b_d3", "b_o1", "b_o2", "b_o3")))
    W1cat, W2cat, W3cat, bias1, bias2, b3rep = wargs
    in_maps = []
    for c in range(n_cores):
        xp, xe = _pack_x(x[c * S:(c + 1) * S])
        in_maps.append({
            "xp": xp, "xe": xe, "w1": W1cat, "w2": W2cat, "w3": W3cat,
            "b1": bias1, "b2": bias2, "b3": b3rep,
        })
    res = run_bass_kernel_spmd(nc, in_maps, list(range(n_cores)), trace=trace)
    outs = [_unpack_out(res.results[c]["out"]) for c in range(n_cores)]
    return np.concatenate(outs, axis=0), res


def kernel(**inputs) -> np.ndarray:
    out, _ = run(inputs, trace=False)
    return out


# revision 19
# speedup vs baseline: 1.0000x; 1.0000x over previous
"""Trainium2 Bass kernel for nn_Damping (tiny 2->16->16->{2,1} tanh MLPs + 2x2 LL^T).

Strategy: pure data parallel over 8 cores (batch split). Inside each core:
  - x is host-packed into a "4-slot" layout: tiles of [8, 512] where rows are
    (slot s, coord c) and each slot covers 512 consecutive samples. This lets
    layers 1/2 run as block-diagonal matmuls producing [128, 512] tiles
    (4 slots x 32 features) that keep the ACT/DVE lanes full.
  - Matmul operands are fp16 (10-bit mantissa, same as TF32): 1 cycle/row on
    the PE (fp32 matmuls lower to 2 half-rate passes = 4x slower) and 2-byte
    fast weight loads. PSUM accumulation stays fp32; measured rel l2 error
    vs the fp32 reference is ~6e-4.
  - tanh+bias is fused on the scalar engine reading PSUM directly, one
    [128, 1024] ACTIVATE per layer per 2-tile chunk (amortizes the ~352-cycle
    per-instruction overhead; PSUM: h1p 2x2 banks, h2p 2, y 2).
  - Layer 3 is "flipped": lhsT = h2 chunk (stationary), rhs = packed w3, so
    the output lands batch-major in PSUM. A rank-1 bias matmul opens each
    4-chunk "quad" accumulation group (start=True clears the bank's
    has_written bits before the 32 layer-3 matmuls accumulate in).
  - The relu/damping/LL^T epilogue runs once per quad (16384 samples) on the
    vector engine with [128, 128] strided views.
Host packs/unpacks with pure numpy (layout marshalling only; all math on HW).
Measured on trn2: ~345 us/core HW exec for 4M samples (8 cores).
"""

import numpy as np
from contextlib import ExitStack

import concourse.bass as bass
import concourse.bacc as bacc
import concourse.tile as tile
import concourse.mybir as mybir
from concourse.bass_utils import run_bass_kernel_spmd

F32 = mybir.dt.float32
F32R = mybir.dt.float32r
F16 = mybir.dt.float16
AF = mybir.ActivationFunctionType
ALU = mybir.AluOpType

NCORES = 8
N = 512            # samples per slot per tile (one PSUM bank)
TS = 4 * N         # samples per tile (4 slots stacked -> 128 partitions)
CHUNK = 2          # tiles per supergroup (PSUM: 2x2+2+2 banks)
DAMP = 1e-3

_programs = {}


def _chunks(G):
    out = []
    g = 0
    while g < G:
        n = min(CHUNK, G - g)
        out.append((g, n))
        g += n
    return out


def build_program(S):
    """One-core program; SPMD across cores via per-core inputs.

    Loop structure: quads of 4 chunks (chunk = 2 tiles of [8,512] -> 4096
    samples). PSUM: h1p [128,1024] bufs=2 (4 banks) + h2p bufs=1 (2 banks) +
    y-quad [128,384] bufs=2 (2 banks). The layer-3 bias matmul opens each
    quad's accumulation group (start=True clears the bank), the 32 flipped
    layer-3 matmuls accumulate into it, and the epilogue+store run once per
    quad on full-width [128,128] views.
    """
    G = S // TS            # [8,512] tiles
    NQ = G // 8            # quads (8 tiles each)
    assert S % TS == 0 and G % 8 == 0
    nc = bacc.Bacc(None, target_bir_lowering=False)
    xp_d = nc.declare_dram_parameter("xp", [8, G * N], F16, isOutput=False)
    xe_d = nc.declare_dram_parameter("xe", [128, G, 4, 4, 2], F32, isOutput=False)
    w1_d = nc.declare_dram_parameter("w1", [8, 128], F16, isOutput=False)
    w2_d = nc.declare_dram_parameter("w2", [128, 128], F16, isOutput=False)
    w3_d = nc.declare_dram_parameter("w3", [128, 12], F16, isOutput=False)
    b1_d = nc.declare_dram_parameter("b1", [128, 1], F32, isOutput=False)
    b2_d = nc.declare_dram_parameter("b2", [128, 1], F32, isOutput=False)
    b3_d = nc.declare_dram_parameter("b3", [1, 384], F16, isOutput=False)
    out_d = nc.declare_dram_parameter("out", [128, G, 4, 4, 2], F32, isOutput=True)

    with tile.TileContext(nc) as tc:
        with ExitStack() as ctx:
            consts = ctx.enter_context(tc.tile_pool(name="consts", bufs=1))
            w1 = consts.tile([8, 128], F16)
            w2 = consts.tile([128, 128], F16)
            w3 = consts.tile([128, 12], F16)
            b1 = consts.tile([128, 1], F32)
            b2 = consts.tile([128, 1], F32)
            b3 = consts.tile([1, 384], F16)
            ones = consts.tile([1, 128], F16)
            nc.sync.dma_start(w1[:], w1_d[:])
            nc.sync.dma_start(w2[:], w2_d[:])
            nc.sync.dma_start(w3[:], w3_d[:])
            nc.sync.dma_start(b1[:], b1_d[:])
            nc.sync.dma_start(b2[:], b2_d[:])
            nc.sync.dma_start(b3[:], b3_d[:])
            nc.vector.memset(ones[:], 1.0)

            xp_pool = ctx.enter_context(tc.tile_pool(name="xp", bufs=3))
            xe_pool = ctx.enter_context(tc.tile_pool(name="xe", bufs=3))
            h_pool = ctx.enter_context(tc.tile_pool(name="h", bufs=4))
            o_pool = ctx.enter_context(tc.tile_pool(name="o", bufs=3))
            t_pool = ctx.enter_context(tc.tile_pool(name="t", bufs=3))
            ps1 = ctx.enter_context(
                tc.tile_pool(name="ps1", bufs=2, space=bass.MemorySpace.PSUM))
            ps2 = ctx.enter_context(
                tc.tile_pool(name="ps2", bufs=1, space=bass.MemorySpace.PSUM))
            psy = ctx.enter_context(
                tc.tile_pool(name="psy", bufs=2, space=bass.MemorySpace.PSUM))

            NC = 4 * NQ                       # chunks of 2 tiles
            quad_res = {}
            for c in range(NC + 1):
                if c < NC:
                    q, qc = divmod(c, 4)
                    if qc == 0:
                        g0 = q * 8
                        xp_t = xp_pool.tile([8, 8 * N], F16, tag="xp")
                        nc.sync.dma_start(xp_t[:], xp_d[:, g0 * N:(g0 + 8) * N])
                        xe_t = xe_pool.tile([128, 8, 4, 4, 2], F32, tag="xe")
                        nc.sync.dma_start(xe_t[:], xe_d[:, g0:g0 + 8])
                        y = psy.tile([128, 8, 4, 4, 3], F32, tag="y")
                        # opens the quad's accumulation group: clears the
                        # bank's has_written bits, deposits layer-3 biases
                        nc.tensor.matmul(y[:], ones[:], b3[:], start=True,
                                         stop=False, skip_group_check=True)
                        quad_res[q] = (xp_t, xe_t, y)
                    xp_t, _, _ = quad_res[q]
                    c0 = 2 * qc
                    h1p = ps1.tile([128, 2 * N], F32, tag="h1p")
                    for t in range(2):
                        nc.tensor.matmul(h1p[:, t * N:(t + 1) * N], w1[:],
                                         xp_t[:, (c0 + t) * N:(c0 + t + 1) * N],
                                         start=True, stop=True)
                    pending = (c, h1p)
                if c >= 1:
                    cp, h1p_p = prev_pending
                    qp, qcp = divmod(cp, 4)
                    _, xe_p, y_p = quad_res[qp]
                    c0 = 2 * qcp
                    h1 = h_pool.tile([128, 2 * N], F16, tag="h1")
                    nc.scalar.activation(h1[:], h1p_p[:], AF.Tanh, bias=b1[:])
                    h2p = ps2.tile([128, 2 * N], F32, tag="h2p")
                    for t in range(2):
                        nc.tensor.matmul(h2p[:, t * N:(t + 1) * N], w2[:],
                                         h1[:, t * N:(t + 1) * N],
                                         start=True, stop=True)
                    h2 = h_pool.tile([128, 2 * N], F16, tag="h2")
                    nc.scalar.activation(h2[:], h2p[:], AF.Tanh, bias=b2[:])
                    for ck in range(8):       # flipped layer 3
                        t, k = divmod(ck, 4)
                        nc.tensor.matmul(y_p[:, c0 + t, k],
                                         h2[:, ck * 128:(ck + 1) * 128],
                                         w3[:], start=False,
                                         stop=(qcp == 3 and ck == 7),
                                         skip_group_check=True)
                    if qcp == 3:              # quad complete: epilogue + store
                        A = y_p[:, :, :, :, 0]
                        Bv = y_p[:, :, :, :, 1]
                        C = y_p[:, :, :, :, 2]
                        X0 = xe_p[:, :, :, :, 0]
                        X1 = xe_p[:, :, :, :, 1]
                        out_t = o_pool.tile([128, 8, 4, 4, 2], F32, tag="out")
                        D0 = out_t[:, :, :, :, 0]
                        D1 = out_t[:, :, :, :, 1]

                        def tmp(tag):
                            return t_pool.tile([128, 8, 4, 4], F32, tag=tag,
                                               name=tag)

                        ra = tmp("ra")
                        nc.vector.tensor_scalar(ra, A, DAMP, DAMP, ALU.add, ALU.max)
                        rb = tmp("rb")
                        nc.vector.tensor_scalar(rb, Bv, DAMP, DAMP, ALU.add, ALU.max)
                        av = tmp("av")
                        nc.vector.tensor_tensor(av, ra, X0, ALU.mult)
                        bv = tmp("bv")
                        nc.vector.tensor_tensor(bv, rb, X1, ALU.mult)
                        u = tmp("u")
                        nc.vector.tensor_tensor(u, av, X0, ALU.mult)
                        v = tmp("v")
                        nc.vector.tensor_tensor(v, C, X1, ALU.mult)
                        w = tmp("w")
                        nc.vector.tensor_tensor(w, u, v, ALU.add)
                        nc.vector.tensor_tensor(D0, av, w, ALU.mult)
                        q_ = tmp("q_")
                        nc.vector.tensor_tensor(q_, bv, bv, ALU.mult)
                        r = tmp("r")
                        nc.vector.tensor_tensor(r, q_, X1, ALU.mult)
                        p1 = tmp("p1")
                        nc.vector.tensor_tensor(p1, C, w, ALU.mult)
                        nc.vector.tensor_tensor(D1, p1, r, ALU.add)
                        nc.sync.dma_start(out_d[:, qp * 8:qp * 8 + 8], out_t[:])
                        del quad_res[qp]
                prev_pending = pending if c < NC else None

    nc.compile()
    return nc


def _get_program(S):
    if S not in _programs:
        _programs[S] = build_program(S)
    return _programs[S]


def _pack_weights(w_d1, w_d2, w_d3, w_o1, w_o2, w_o3,
                  b_d1, b_d2, b_d3, b_o1, b_o2, b_o3):
    W1 = np.concatenate([w_d1, w_o1], axis=1)          # [2, 32]
    W1cat = np.zeros((8, 128), np.float32)
    W2sub = np.zeros((32, 32), np.float32)
    W2sub[:16, :16] = w_d2
    W2sub[16:, 16:] = w_o2
    W2cat = np.zeros((128, 128), np.float32)
    W3sub = np.zeros((32, 3), np.float32)
    W3sub[:16, :2] = w_d3
    W3sub[16:, 2] = w_o3[:, 0]
    W3cat = np.zeros((128, 12), np.float32)
    for s in range(4):
        W1cat[2 * s:2 * s + 2, 32 * s:32 * s + 32] = W1
        W2cat[32 * s:32 * s + 32, 32 * s:32 * s + 32] = W2sub
        W3cat[32 * s:32 * s + 32, 3 * s:3 * s + 3] = W3sub
    bias1 = np.tile(np.concatenate([b_d1, b_o1]), 4).astype(np.float32)[:, None]
    bias2 = np.tile(np.concatenate([b_d2, b_o2]), 4).astype(np.float32)[:, None]
    b3 = np.array([b_d3[0], b_d3[1], b_o3[0]], np.float32)
    b3rep = np.tile(b3, 128)[None, :].astype(np.float16)
    return (W1cat.astype(np.float16), W2cat.astype(np.float16),
            W3cat.astype(np.float16), bias1, bias2, b3rep)


def _pack_x(xc):
    """xc: [S, 2] -> (xp [8, G*N], xe [128, G, 4, 4, 2])."""
    S = xc.shape[0]
    G = S // TS
    xp = np.ascontiguousarray(
        xc.reshape(G, 4, N, 2).transpose(1, 3, 0, 2).reshape(8, G * N)
        .astype(np.float16))
    xe = np.ascontiguousarray(
        xc.reshape(G, 4, 4, 128, 2).transpose(3, 0, 2, 1, 4))
    return xp, xe


def _unpack_out(o):
    """o: [128, G, 4(k), 4(s), 2] -> [S, 2]."""
    G = o.shape[1]
    return np.ascontiguousarray(
        o.transpose(1, 3, 2, 0, 4).reshape(G * TS, 2))


def run(inputs, trace=False, n_cores=NCORES):
    x = np.ascontiguousarray(np.asarray(inputs["x"], np.float32))
    B = x.shape[0]
    S = B // n_cores
    nc = _get_program(S)
    wargs = _pack_weights(
        *(np.asarray(inputs[k], np.float32) for k in
          ("w_d1", "w_d2", "w_d3", "w_o1", "w_o2", "w_o3",
           "b_d1", "b_d2", "b_d3", "b_o1", "b_o2", "b_o3")))
    W1cat, W2cat, W3cat, bias1, bias2, b3rep = wargs
    in_maps = []
    for c in range(n_cores):
        xp, xe = _pack_x(x[c * S:(c + 1) * S])
        in_maps.append({
            "xp": xp, "xe": xe, "w1": W1cat, "w2": W2cat, "w3": W3cat,
            "b1": bias1, "b2": bias2, "b3": b3rep,
        })
    res = run_bass_kernel_spmd(nc, in_maps, list(range(n_cores)), trace=trace)
    outs = [_unpack_out(res.results[c]["out"]) for c in range(n_cores)]
    return np.concatenate(outs, axis=0), res


def kernel(**inputs) -> np.ndarray:
    out, _ = run(inputs, trace=False)
    return out
